# revision 9
# baseline (speedup 1.0000x reference)
"""Mode-adaptive linear (MoE soft routing) Trainium2 kernel.

out[b, o] = sum_c weights[b, c] * (inputs[b, :] @ w[c])[o] + (weights @ bias)[b, o]

Strategy: data-parallel shard of the batch across 8 NeuronCores (1024 rows
each); w/bias replicated.  On each core the routing weights are folded into
the transposed input tiles (xs_c = weights[:, c] * X^T in bf16), so all 8
expert matmuls plus the bias matmul accumulate into a single PSUM bank per
128-row batch tile — equivalent to one K=4104 matmul per tile.
"""

import json
import types

import numpy as np

import concourse.bass as bass
import concourse.mybir as mybir
import concourse.tile as tile
from concourse.bass import ts
from concourse.bass_utils import run_bass_kernel_spmd

N_CORES = 8
B, D_IN, D_OUT, N_CTRL = 8192, 512, 512, 8
B_SHARD = B // N_CORES          # 1024 rows per core
P = 128
N_TILES = B_SHARD // P          # 8 batch tiles per core
KS = D_IN // P                  # 4 K-chunks of 128
F32 = mybir.dt.float32
BF16 = mybir.dt.bfloat16


def _consts(nc: bass.Bass, const_pool):
    """One-time constants, embedded in the NEFF and DMA'd to SBUF (no engine
    work): identity for PE transpose, and the expert-selection matrix
    e_pad[p, c, m] = 1 iff p == c, so matmul(lhsT=e_pad[:, c], rhs=wt_pad)
    broadcasts wt_pad row c to all 128 output partitions."""
    import ml_dtypes

    identity_d = nc.inline_tensor(np.eye(P, dtype=np.float32), name="identity_const")
    identity = const_pool.tile([P, P], F32)
    nc.sync.dma_start(identity, identity_d.ap())

    # e_pad's DMA is deferred into _body (after the first x tiles) so it
    # doesn't delay the first PE transpose; only the handle is created here.
    e_np = np.zeros((P, N_CTRL, P), dtype=ml_dtypes.bfloat16)
    for c in range(N_CTRL):
        e_np[c, c, :] = 1.0
    e_d = nc.inline_tensor(e_np, name="e_pad_const")
    e_pad = const_pool.tile([P, N_CTRL, P], BF16)
    return identity, (e_pad, e_d)


def _body(nc: bass.Bass, tc: tile.TileContext, x_d, wt_d, w_d, b_d, o_d,
          identity, e_pad_pair):
    e_pad, e_d = e_pad_pair
    with (
        tc.tile_pool(name="const", bufs=1) as const_pool,
        tc.tile_pool(name="wstage", bufs=2) as wstage,
        tc.tile_pool(name="xpool", bufs=4) as xpool,
        tc.tile_pool(name="xtpool", bufs=N_TILES) as xtpool,
        tc.tile_pool(name="xspool", bufs=3) as xspool,
        tc.tile_pool(name="opool", bufs=3) as opool,
        tc.tile_pool(name="tr_ps", bufs=2, space="PSUM") as tr_psum,
        tc.tile_pool(name="mm_ps", bufs=4, space="PSUM") as mm_psum,
    ):
        bc_psum = mm_psum  # share banks: bc only used during setup

        # --- Phase 1: small loads + everything not needing the 8MB w ---

        # First two x tiles right away so PE transposes can start ASAP.
        x_f32s = []
        for t in range(2):
            x_f32 = xpool.tile([P, D_IN], F32, tag="x_f32")
            nc.sync.dma_start(x_f32, x_d[ts(t, P), :])
            x_f32s.append(x_f32)

        # Selection-matrix constant (inline) — issued after the x tiles so it
        # doesn't delay the first PE transpose.
        nc.sync.dma_start(e_pad, e_d.ap())

        # HAM warmup: ~16 dummy bf16 matmuls fill idle PE slots during the
        # DMA-paced startup so the PE clock-gate (4/8 cold -> 8/8 warm after
        # ~3.4us of sustained activity) is released before the real matmul
        # stream begins.  Results are never read.
        for _ in range(12):
            warm_ps = tr_psum.tile([P, P], F32, tag="tr_ps", name="warm_ps")
            nc.tensor.matmul(
                warm_ps, lhsT=e_pad[:, 0, :], rhs=e_pad[:, 0, :],
                start=True, stop=True,
            )

        # Routing weights: load naturally as [128, tile, 8] (32B runs), then
        # one PE transpose per tile gives wt^T [8, 128] chunks, padded to 128
        # partitions in bf16 (rows 8..127 = 0 so wt_pad can serve as a K=128
        # matmul operand).
        wt_nat = const_pool.tile([P, N_TILES, N_CTRL], F32)
        nc.sync.dma_start(wt_nat, wt_d.rearrange("(t p) c -> p t c", p=P))
        wt_pad = const_pool.tile([P, B_SHARD], BF16)
        nc.gpsimd.memset(wt_pad, 0.0)

        # X^T per tile via PE transpose (cast to bf16 on ScalarE), with the
        # Wb broadcast matmuls interleaved after the first two tiles so PE
        # has work while later x tiles stream in.
        # Wb[p, c, b] = weights[b, c] for every partition p.
        wb = const_pool.tile([P, N_CTRL, B_SHARD], BF16)
        xts = []

        def transpose_tile(t):
            if t < 2:
                x_f32 = x_f32s[t]
            else:
                x_f32 = xpool.tile([P, D_IN], F32, tag="x_f32")
                nc.sync.dma_start(x_f32, x_d[ts(t, P), :])
            tr_ps = tr_psum.tile([P, KS, P], F32)
            xt = xtpool.tile([P, KS, P], BF16)
            for k in range(KS):
                nc.tensor.transpose(tr_ps[:, k, :], x_f32[:, ts(k, P)], identity)
            nc.scalar.copy(xt, tr_ps)
            xts.append(xt)

        for t in range(2):
            transpose_tile(t)

        for t in range(N_TILES):
            wtt_ps = mm_psum.tile([N_CTRL, P], F32, tag="bc", bufs=2)
            nc.tensor.transpose(wtt_ps, wt_nat[:, t, :], identity)
            nc.scalar.copy(wt_pad[0:N_CTRL, ts(t, P)], wtt_ps)

        # Bias, zero-padded the same way.
        b_f32 = const_pool.tile([N_CTRL, D_OUT], F32)
        nc.sync.dma_start(b_f32, b_d)
        b_pad = const_pool.tile([P, D_OUT], BF16)
        nc.gpsimd.memset(b_pad, 0.0)
        nc.gpsimd.tensor_copy(b_pad[0:N_CTRL, :], b_f32)

        for c in range(N_CTRL):
            for h in range(B_SHARD // 512):
                bc_ps = bc_psum.tile([P, 512], F32, tag="bc", bufs=2)
                nc.tensor.matmul(
                    bc_ps,
                    lhsT=e_pad[:, c, :],
                    rhs=wt_pad[:, ts(h, 512)],
                    start=True,
                    stop=True,
                )
                nc.scalar.copy(wb[:, c, ts(h, 512)], bc_ps)

        # --- Phase 2: bulk expert-weight load + cast, one DMA per (expert,
        # K-chunk).  The first expert group's chunks are issued before the
        # remaining x tiles so the matmul stream is never starved. ---
        # [128 (i%128), expert, i//128, o] in bf16.
        w_sb = const_pool.tile([P, N_CTRL, KS, D_OUT], BF16)

        def load_w(c):
            for k in range(KS):
                w_f32 = wstage.tile([P, D_OUT], F32, tag="w_f32", bufs=8)
                nc.sync.dma_start(w_f32, w_d[c, ts(k, P), :])
                nc.gpsimd.tensor_copy(w_sb[:, c, k], w_f32)

        load_w(0)
        load_w(1)
        for t in range(2, N_TILES):
            transpose_tile(t)
            if t - 2 < 6:
                load_w(t)  # experts 2..7 behind tiles 2..7
        # (experts 2..7 all covered by the loop above since N_TILES-2 == 6)

        # --- Phase 3: scale + matmul-accumulate, experts in 2 groups of 4 so
        # PE only ever waits on the first half of the w load; partial sums
        # chain through an SBUF accumulator per tile. ---
        GROUPS = 2
        CPG = N_CTRL // GROUPS
        accs = [None] * N_TILES
        for g in range(GROUPS):
            for t in range(N_TILES):
                # Fold routing weights in: xs[:, ci] = X^T * weights[:, c]
                # — one DVE op for the whole expert group (bf16 2x).
                xs = xspool.tile([P, CPG, KS, P], BF16)
                nc.vector.tensor_mul(
                    xs,
                    xts[t][:, None, :, :].to_broadcast([P, CPG, KS, P]),
                    wb[:, ts(g, CPG), None, ts(t, P)].to_broadcast(
                        [P, CPG, KS, P]
                    ),
                )

                out_ps = mm_psum.tile([P, D_OUT], F32, tag="acc")
                if g == 0:
                    # Bias rides in the first group's accumulation.
                    nc.tensor.matmul(
                        out_ps,
                        lhsT=wt_pad[:, ts(t, P)],
                        rhs=b_pad,
                        start=True,
                        stop=False,
                    )
                for ci in range(CPG):
                    c = g * CPG + ci
                    for k in range(KS):
                        nc.tensor.matmul(
                            out_ps,
                            lhsT=xs[:, ci, k, :],
                            rhs=w_sb[:, c, k, :],
                            start=(g != 0 and ci == 0 and k == 0),
                            stop=(ci == CPG - 1 and k == KS - 1),
                        )

                if g == 0:
                    acc = opool.tile([P, D_OUT], F32, tag="acc_sb", bufs=N_TILES)
                    nc.scalar.copy(acc, out_ps)
                    accs[t] = acc
                elif g < GROUPS - 1:
                    # acc += psum (in-place on DVE)
                    nc.vector.tensor_add(accs[t], out_ps, accs[t])
                else:
                    o_sb = opool.tile([P, D_OUT], F32, tag="o_sb")
                    nc.vector.tensor_add(o_sb, out_ps, accs[t])
                    nc.sync.dma_start(o_d[ts(t, P), :], o_sb)


def _split_multi_waits(bir: dict) -> dict:
    """The walrus build in this container supports at most ONE sync-wait per
    instruction ("Too many sync wait commands" at codegen otherwise).  Tile's
    scheduler freely attaches several.  Split: keep the last wait on the
    instruction and hoist the others onto standalone same-engine
    EventSemaphore instructions inserted immediately before it — identical
    semantics (the engine blocks at the same program point)."""
    ctr = 0
    for func in bir["functions"]:
        for bb in func["blocks"]:
            new_insts = []
            for inst in bb["instructions"]:
                si = inst.get("sync_info")
                waits = si.get("on_wait") if si else None
                if waits and len(waits) > 1:
                    for w in waits[:-1]:
                        ctr += 1
                        new_insts.append(
                            {
                                "debug": inst.get("debug", 0),
                                "engine": inst["engine"],
                                "ins": [],
                                "outs": [],
                                "name": f"{inst['name']}-wsplit{ctr}",
                                "opcode": "EventSemaphore",
                                "sync_info": {"on_update": [], "on_wait": [w]},
                            }
                        )
                    si["on_wait"] = [waits[-1]]
                new_insts.append(inst)
            bb["instructions"] = new_insts
    return bir


_ORIG_TO_JSON_BYTES = bass.Bass.to_json_bytes


def _patched_to_json_bytes(self) -> bytes:
    bir = json.loads(_ORIG_TO_JSON_BYTES(self))
    _split_multi_waits(bir)
    return json.dumps(bir).encode()


_NC_CACHE = {}


def _build(reps: int = 1) -> bass.Bass:
    if reps in _NC_CACHE:
        return _NC_CACHE[reps]
    nc = bass.Bass(
        "TRN2",
        target_bir_lowering=False,
        debug=False,
        enable_asserts=False,
        num_devices=N_CORES,
    )
    x_d = nc.dram_tensor("x_in", [B_SHARD, D_IN], F32, kind="ExternalInput").ap()
    wt_d = nc.dram_tensor("wt_in", [B_SHARD, N_CTRL], F32, kind="ExternalInput").ap()
    w_d = nc.dram_tensor("w_in", [N_CTRL, D_IN, D_OUT], F32, kind="ExternalInput").ap()
    b_d = nc.dram_tensor("b_in", [N_CTRL, D_OUT], F32, kind="ExternalInput").ap()
    o_d = nc.dram_tensor("out", [B_SHARD, D_OUT], F32, kind="ExternalOutput").ap()
    with tile.TileContext(nc) as tc:
        with tc.tile_pool(name="global_const", bufs=1) as gconst:
            identity, e_pad = _consts(nc, gconst)
            for _ in range(reps):
                _body(nc, tc, x_d, wt_d, w_d, b_d, o_d, identity, e_pad)
    nc.to_json_bytes = types.MethodType(_patched_to_json_bytes, nc)
    _NC_CACHE[reps] = nc
    return nc


def kernel(inputs, weights, w, b, _trace=False):
    nc = _build()
    inputs = np.ascontiguousarray(inputs, dtype=np.float32)
    weights = np.ascontiguousarray(weights, dtype=np.float32)
    w = np.ascontiguousarray(w, dtype=np.float32)
    b = np.ascontiguousarray(b, dtype=np.float32)

    in_maps = []
    for i in range(N_CORES):
        sl = slice(i * B_SHARD, (i + 1) * B_SHARD)
        in_maps.append(
            {
                "x_in": inputs[sl],
                "wt_in": weights[sl],
                "w_in": w,
                "b_in": b,
            }
        )
    res = run_bass_kernel_spmd(
        nc, in_maps, core_ids=list(range(N_CORES)), trace=_trace
    )
    out = np.concatenate([r["out"] for r in res.results], axis=0)
    if _trace:
        return out, res
    return out



# revision 10
# speedup vs baseline: 1.0692x; 1.0692x over previous
"""Mode-adaptive linear (MoE soft routing) Trainium2 kernel.

out[b, o] = sum_c weights[b, c] * (inputs[b, :] @ w[c])[o] + (weights @ bias)[b, o]

Strategy: data-parallel shard of the batch across 8 NeuronCores (1024 rows
each); w/bias replicated.  On each core the routing weights are folded into
the transposed input tiles (xs_c = weights[:, c] * X^T in bf16), so all 8
expert matmuls plus the bias matmul accumulate into a single PSUM bank per
128-row batch tile — equivalent to one K=4104 matmul per tile.
"""

import json
import types

import numpy as np

import concourse.bass as bass
import concourse.mybir as mybir
import concourse.tile as tile
from concourse.bass import ts
from concourse.bass_utils import run_bass_kernel_spmd

N_CORES = 8
B, D_IN, D_OUT, N_CTRL = 8192, 512, 512, 8
B_SHARD = B // N_CORES          # 1024 rows per core
P = 128
N_TILES = B_SHARD // P          # 8 batch tiles per core
KS = D_IN // P                  # 4 K-chunks of 128
F32 = mybir.dt.float32
BF16 = mybir.dt.bfloat16


def _consts(nc: bass.Bass, const_pool):
    """One-time constants, embedded in the NEFF and DMA'd to SBUF (no engine
    work): identity for PE transpose, and the expert-selection matrix
    e_pad[p, c, m] = 1 iff p == c, so matmul(lhsT=e_pad[:, c], rhs=wt_pad)
    broadcasts wt_pad row c to all 128 output partitions."""
    import ml_dtypes

    identity_d = nc.inline_tensor(np.eye(P, dtype=np.float32), name="identity_const")
    identity = const_pool.tile([P, P], F32)
    nc.sync.dma_start(identity, identity_d.ap())

    # e_pad's DMA is deferred into _body (after the first x tiles) so it
    # doesn't delay the first PE transpose; only the handle is created here.
    e_np = np.zeros((P, N_CTRL, P), dtype=ml_dtypes.bfloat16)
    for c in range(N_CTRL):
        e_np[c, c, :] = 1.0
    e_d = nc.inline_tensor(e_np, name="e_pad_const")
    e_pad = const_pool.tile([P, N_CTRL, P], BF16)
    return identity, (e_pad, e_d)


def _body(nc: bass.Bass, tc: tile.TileContext, x_d, wt_d, w_d, b_d, o_d,
          identity, e_pad_pair):
    e_pad, e_d = e_pad_pair
    with (
        tc.tile_pool(name="const", bufs=1) as const_pool,
        tc.tile_pool(name="wstage", bufs=2) as wstage,
        tc.tile_pool(name="xpool", bufs=4) as xpool,
        tc.tile_pool(name="xtpool", bufs=N_TILES) as xtpool,
        tc.tile_pool(name="xspool", bufs=3) as xspool,
        tc.tile_pool(name="opool", bufs=3) as opool,
        tc.tile_pool(name="tr_ps", bufs=2, space="PSUM") as tr_psum,
        tc.tile_pool(name="mm_ps", bufs=4, space="PSUM") as mm_psum,
    ):
        bc_psum = mm_psum  # share banks: bc only used during setup

        # --- Phase 1: small loads + everything not needing the 8MB w ---

        # First two x tiles right away so PE transposes can start ASAP.
        x_f32s = []
        for t in range(2):
            x_f32 = xpool.tile([P, D_IN], F32, tag="x_f32")
            nc.sync.dma_start(x_f32, x_d[ts(t, P), :])
            x_f32s.append(x_f32)

        # Selection-matrix constant (inline) — issued after the x tiles so it
        # doesn't delay the first PE transpose.
        nc.sync.dma_start(e_pad, e_d.ap())

        # HAM warmup: ~16 dummy bf16 matmuls fill idle PE slots during the
        # DMA-paced startup so the PE clock-gate (4/8 cold -> 8/8 warm after
        # ~3.4us of sustained activity) is released before the real matmul
        # stream begins.  Results are never read.
        for _ in range(12):
            warm_ps = tr_psum.tile([P, P], F32, tag="tr_ps", name="warm_ps")
            nc.tensor.matmul(
                warm_ps, lhsT=e_pad[:, 0, :], rhs=e_pad[:, 0, :],
                start=True, stop=True,
            )

        # Routing weights: load naturally as [128, tile, 8] (32B runs), then
        # one PE transpose per tile gives wt^T [8, 128] chunks, padded to 128
        # partitions in bf16 (rows 8..127 = 0 so wt_pad can serve as a K=128
        # matmul operand).
        wt_nat = const_pool.tile([P, N_TILES, N_CTRL], F32)
        nc.sync.dma_start(wt_nat, wt_d.rearrange("(t p) c -> p t c", p=P))
        wt_pad = const_pool.tile([P, B_SHARD], BF16)
        nc.gpsimd.memset(wt_pad, 0.0)

        # X^T per tile via PE transpose (cast to bf16 on ScalarE), with the
        # Wb broadcast matmuls interleaved after the first two tiles so PE
        # has work while later x tiles stream in.
        # Wb[p, c, b] = weights[b, c] for every partition p.
        wb = const_pool.tile([P, N_CTRL, B_SHARD], BF16)
        xts = []

        def transpose_tile(t):
            if t < 2:
                x_f32 = x_f32s[t]
            else:
                x_f32 = xpool.tile([P, D_IN], F32, tag="x_f32")
                nc.sync.dma_start(x_f32, x_d[ts(t, P), :])
            tr_ps = tr_psum.tile([P, KS, P], F32)
            xt = xtpool.tile([P, KS, P], BF16)
            for k in range(KS):
                nc.tensor.transpose(tr_ps[:, k, :], x_f32[:, ts(k, P)], identity)
            nc.scalar.copy(xt, tr_ps)
            xts.append(xt)

        for t in range(2):
            transpose_tile(t)

        for t in range(N_TILES):
            wtt_ps = mm_psum.tile([N_CTRL, P], F32, tag="bc", bufs=2)
            nc.tensor.transpose(wtt_ps, wt_nat[:, t, :], identity)
            nc.scalar.copy(wt_pad[0:N_CTRL, ts(t, P)], wtt_ps)

        # Bias, zero-padded the same way.
        b_f32 = const_pool.tile([N_CTRL, D_OUT], F32)
        nc.sync.dma_start(b_f32, b_d)
        b_pad = const_pool.tile([P, D_OUT], BF16)
        nc.gpsimd.memset(b_pad, 0.0)
        nc.vector.tensor_copy(b_pad[0:N_CTRL, :], b_f32)

        for c in range(N_CTRL):
            for h in range(B_SHARD // 512):
                bc_ps = bc_psum.tile([P, 512], F32, tag="bc", bufs=2)
                nc.tensor.matmul(
                    bc_ps,
                    lhsT=e_pad[:, c, :],
                    rhs=wt_pad[:, ts(h, 512)],
                    start=True,
                    stop=True,
                )
                nc.scalar.copy(wb[:, c, ts(h, 512)], bc_ps)

        # --- Phase 2: bulk expert-weight load + cast, one DMA per (expert,
        # K-chunk).  The first expert group's chunks are issued before the
        # remaining x tiles so the matmul stream is never starved. ---
        # [128 (i%128), expert, i//128, o] in bf16.
        w_sb = const_pool.tile([P, N_CTRL, KS, D_OUT], BF16)

        def load_w(c):
            for k in range(KS):
                w_f32 = wstage.tile([P, D_OUT], F32, tag="w_f32", bufs=8)
                nc.sync.dma_start(w_f32, w_d[c, ts(k, P), :])
                nc.scalar.copy(w_sb[:, c, k], w_f32)

        load_w(0)
        load_w(1)
        for t in range(2, N_TILES):
            transpose_tile(t)
            if t - 2 < 6:
                load_w(t)  # experts 2..7 behind tiles 2..7
        # (experts 2..7 all covered by the loop above since N_TILES-2 == 6)

        # --- Phase 3: scale + matmul-accumulate, experts in 2 groups of 4 so
        # PE only ever waits on the first half of the w load; partial sums
        # chain through an SBUF accumulator per tile. ---
        GROUPS = 2
        CPG = N_CTRL // GROUPS
        accs = [None] * N_TILES
        for g in range(GROUPS):
            for t in range(N_TILES):
                # Fold routing weights in: xs[:, ci] = X^T * weights[:, c]
                # — one DVE op for the whole expert group (bf16 2x).
                xs = xspool.tile([P, CPG, KS, P], BF16)
                nc.vector.tensor_mul(
                    xs,
                    xts[t][:, None, :, :].to_broadcast([P, CPG, KS, P]),
                    wb[:, ts(g, CPG), None, ts(t, P)].to_broadcast(
                        [P, CPG, KS, P]
                    ),
                )

                out_ps = mm_psum.tile([P, D_OUT], F32, tag="acc")
                if g == 0:
                    # Bias rides in the first group's accumulation.
                    nc.tensor.matmul(
                        out_ps,
                        lhsT=wt_pad[:, ts(t, P)],
                        rhs=b_pad,
                        start=True,
                        stop=False,
                    )
                for ci in range(CPG):
                    c = g * CPG + ci
                    for k in range(KS):
                        nc.tensor.matmul(
                            out_ps,
                            lhsT=xs[:, ci, k, :],
                            rhs=w_sb[:, c, k, :],
                            start=(g != 0 and ci == 0 and k == 0),
                            stop=(ci == CPG - 1 and k == KS - 1),
                        )

                if g == 0:
                    acc = opool.tile([P, D_OUT], F32, tag="acc_sb", bufs=N_TILES)
                    nc.scalar.copy(acc, out_ps)
                    accs[t] = acc
                elif g < GROUPS - 1:
                    # acc += psum (in-place on DVE)
                    nc.vector.tensor_add(accs[t], out_ps, accs[t])
                else:
                    o_sb = opool.tile([P, D_OUT], F32, tag="o_sb")
                    nc.vector.tensor_add(o_sb, out_ps, accs[t])
                    nc.sync.dma_start(o_d[ts(t, P), :], o_sb)


def _split_multi_waits(bir: dict) -> dict:
    """The walrus build in this container supports at most ONE sync-wait per
    instruction ("Too many sync wait commands" at codegen otherwise).  Tile's
    scheduler freely attaches several.  Split: keep the last wait on the
    instruction and hoist the others onto standalone same-engine
    EventSemaphore instructions inserted immediately before it — identical
    semantics (the engine blocks at the same program point)."""
    ctr = 0
    for func in bir["functions"]:
        for bb in func["blocks"]:
            new_insts = []
            for inst in bb["instructions"]:
                si = inst.get("sync_info")
                waits = si.get("on_wait") if si else None
                if waits and len(waits) > 1:
                    for w in waits[:-1]:
                        ctr += 1
                        new_insts.append(
                            {
                                "debug": inst.get("debug", 0),
                                "engine": inst["engine"],
                                "ins": [],
                                "outs": [],
                                "name": f"{inst['name']}-wsplit{ctr}",
                                "opcode": "EventSemaphore",
                                "sync_info": {"on_update": [], "on_wait": [w]},
                            }
                        )
                    si["on_wait"] = [waits[-1]]
                new_insts.append(inst)
            bb["instructions"] = new_insts
    return bir


_ORIG_TO_JSON_BYTES = bass.Bass.to_json_bytes


def _patched_to_json_bytes(self) -> bytes:
    bir = json.loads(_ORIG_TO_JSON_BYTES(self))
    _split_multi_waits(bir)
    return json.dumps(bir).encode()


_NC_CACHE = {}


def _build(reps: int = 1) -> bass.Bass:
    if reps in _NC_CACHE:
        return _NC_CACHE[reps]
    nc = bass.Bass(
        "TRN2",
        target_bir_lowering=False,
        debug=False,
        enable_asserts=False,
        num_devices=N_CORES,
    )
    x_d = nc.dram_tensor("x_in", [B_SHARD, D_IN], F32, kind="ExternalInput").ap()
    wt_d = nc.dram_tensor("wt_in", [B_SHARD, N_CTRL], F32, kind="ExternalInput").ap()
    w_d = nc.dram_tensor("w_in", [N_CTRL, D_IN, D_OUT], F32, kind="ExternalInput").ap()
    b_d = nc.dram_tensor("b_in", [N_CTRL, D_OUT], F32, kind="ExternalInput").ap()
    o_d = nc.dram_tensor("out", [B_SHARD, D_OUT], F32, kind="ExternalOutput").ap()
    with tile.TileContext(nc) as tc:
        with tc.tile_pool(name="global_const", bufs=1) as gconst:
            identity, e_pad = _consts(nc, gconst)
            for _ in range(reps):
                _body(nc, tc, x_d, wt_d, w_d, b_d, o_d, identity, e_pad)
    nc.to_json_bytes = types.MethodType(_patched_to_json_bytes, nc)
    _NC_CACHE[reps] = nc
    return nc


def kernel(inputs, weights, w, b, _trace=False):
    nc = _build()
    inputs = np.ascontiguousarray(inputs, dtype=np.float32)
    weights = np.ascontiguousarray(weights, dtype=np.float32)
    w = np.ascontiguousarray(w, dtype=np.float32)
    b = np.ascontiguousarray(b, dtype=np.float32)

    in_maps = []
    for i in range(N_CORES):
        sl = slice(i * B_SHARD, (i + 1) * B_SHARD)
        in_maps.append(
            {
                "x_in": inputs[sl],
                "wt_in": weights[sl],
                "w_in": w,
                "b_in": b,
            }
        )
    res = run_bass_kernel_spmd(
        nc, in_maps, core_ids=list(range(N_CORES)), trace=_trace
    )
    out = np.concatenate([r["out"] for r in res.results], axis=0)
    if _trace:
        return out, res
    return out



# revision 14
# speedup vs baseline: 1.2708x; 1.1885x over previous
"""Mode-adaptive linear (MoE soft routing) Trainium2 kernel, v3.

out[b, o] = sum_c weights[b, c] * (inputs[b, :] @ w[c])[o] + (weights @ bias)[b, o]

The 8 axon NeuronCores share one chip's HBM, so the kernel is bound by
AGGREGATE DRAM traffic, not compute.  v3 halves the bytes: all inputs are
staged host-side in bf16 (w: 8MB->4MB per core, x: 2->0.5MB pre-transposed
and tiled, out written in bf16 1MB and upcast on host).  Per core ~5.5MB
instead of 12MB.

On-chip: routing weights folded into the pre-transposed input tiles
(xs_c = weights[:, c] * X^T), bias + all expert matmuls of a group
accumulate in a single PSUM bank per 128-row tile (K=8 bias matmul needs no
padding), partial sums bridge groups through an SBUF accumulator.
"""

import json
import types

import numpy as np

import concourse.bass as bass
import concourse.mybir as mybir
import concourse.tile as tile
from concourse.bass import ts
from concourse.bass_utils import run_bass_kernel_spmd

N_CORES = 8
ABLATE = set()
B, D_IN, D_OUT, N_CTRL = 8192, 512, 512, 8
B_SHARD = B // N_CORES          # 1024 rows per core
P = 128
N_TILES = B_SHARD // P          # 8 batch tiles per core
KS = D_IN // P                  # 4 K-chunks of 128
F32 = mybir.dt.float32
BF16 = mybir.dt.bfloat16


def _consts(nc: bass.Bass, const_pool):
    """e_pad[p, c, m] = 1 iff p == c: matmul(lhsT=e_pad[0:8, c, :], rhs)
    broadcasts rhs row c to all 128 output partitions (K=8)."""
    import ml_dtypes

    e_np = np.zeros((P, N_CTRL, P), dtype=ml_dtypes.bfloat16)
    for c in range(N_CTRL):
        e_np[c, c, :] = 1.0
    e_d = nc.inline_tensor(e_np, name="e_pad_const")
    e_pad = const_pool.tile([P, N_CTRL, P], BF16)
    nc.sync.dma_start(e_pad, e_d.ap())
    return e_pad


def _body(nc: bass.Bass, tc: tile.TileContext, xt_d, wtT_d, w_d, b_d, o_d,
          e_pad, pools):
    if True:
        (const_pool, xtpool, xspool, opool, mm_psum, bc_psum) = pools
        bc_psum = mm_psum
        # --- Phase 1: small loads; first xt tiles; warmup.  DMAs are
        # deliberately COARSE: per-DMA fixed cost (~1.5us descriptor +
        # semaphore) dominates over bytes on this queue. ---
        xts = []

        def load_xt_half(h):
            blk = xtpool.tile([P, 4, KS, P], BF16, tag="xt")
            nc.sync.dma_start(
                blk, xt_d[4 * h:4 * h + 4].rearrange("t p k b -> p t k b")
            )
            for i in range(4):
                xts.append(blk[:, i])

        load_xt_half(0)

        wtT_sb = const_pool.tile([N_CTRL, B_SHARD], BF16)
        nc.sync.dma_start(wtT_sb, wtT_d)
        b_sb = const_pool.tile([N_CTRL, D_OUT], BF16)
        nc.sync.dma_start(b_sb, b_d)

        # HAM warmup: dummy bf16 matmuls keep PE executing during the
        # DMA-paced startup so the clock-gate releases early.
        for _ in range(12):
            warm_ps = bc_psum.tile([P, P], F32, tag="bc", bufs=2, name="warm_ps")
            nc.tensor.matmul(
                warm_ps, lhsT=e_pad[:, 0, :], rhs=e_pad[:, 0, :],
                start=True, stop=True,
            )

        # Wb[p, c, b] = weights[b, c] on every partition p (K=8 broadcast
        # matmuls off wtT rows).
        wb = const_pool.tile([P, N_CTRL, B_SHARD], BF16)
        for c in range(N_CTRL):
            for h in range(B_SHARD // 512):
                bc_ps = bc_psum.tile([P, 512], F32, tag="bc", bufs=2)
                nc.tensor.matmul(
                    bc_ps,
                    lhsT=e_pad[0:N_CTRL, c, :],
                    rhs=wtT_sb[:, ts(h, 512)],
                    start=True,
                    stop=True,
                )
                nc.scalar.copy(wb[:, c, ts(h, 512)], bc_ps)

        # --- Phase 2: expert weights (bf16 in DRAM) straight to SBUF;
        # remaining xt tiles interleaved. ---
        w_sb = const_pool.tile([P, N_CTRL, KS, D_OUT], BF16)

        def load_w(c):
            wr = w_d[c].rearrange("(k p) o -> p k o", p=P)
            nc.sync.dma_start(w_sb[:, c, 0:2], wr[:, 0:2])
            nc.sync.dma_start(w_sb[:, c, 2:4], wr[:, 2:4])

        load_w(0)
        load_w(1)
        load_xt_half(1)
        for c in range(2, N_CTRL):
            load_w(c)

        # --- Phase 3: experts in 2 groups of 4; per (group, tile): one DVE
        # scale op, K=8 bias matmul (group 0) + 16 expert matmuls into one
        # PSUM bank; SBUF accumulator bridges groups; bf16 output. ---
        GROUPS = 2
        CPG = N_CTRL // GROUPS
        accs = [None] * N_TILES
        for g in range(GROUPS):
            for t in range(N_TILES):
                xs = xspool.tile([P, CPG, KS, P], BF16)
                nc.vector.tensor_mul(
                    xs,
                    xts[t][:, None, :, :].to_broadcast([P, CPG, KS, P]),
                    wb[:, ts(g, CPG), None, ts(t, P)].to_broadcast(
                        [P, CPG, KS, P]
                    ),
                )

                out_ps = mm_psum.tile([P, D_OUT], F32, tag="acc")
                if g == 0:
                    # Blended bias rides the first group's accumulation (K=8).
                    nc.tensor.matmul(
                        out_ps,
                        lhsT=wtT_sb[:, ts(t, P)],
                        rhs=b_sb,
                        start=True,
                        stop=False,
                    )
                for ci in range(CPG):
                    c = g * CPG + ci
                    for k in range(KS):
                        nc.tensor.matmul(
                            out_ps,
                            lhsT=xs[:, ci, k, :],
                            rhs=w_sb[:, c, k, :],
                            start=(g != 0 and ci == 0 and k == 0),
                            stop=(ci == CPG - 1 and k == KS - 1),
                        )

                if g == 0:
                    acc = opool.tile([P, D_OUT], F32, tag="acc_sb", bufs=N_TILES)
                    nc.scalar.copy(acc, out_ps)
                    accs[t] = acc
                elif g < GROUPS - 1:
                    nc.vector.tensor_add(accs[t], out_ps, accs[t])
                else:
                    o_sb = opool.tile([P, D_OUT], BF16, tag="o_sb")
                    nc.vector.tensor_add(o_sb, out_ps, accs[t])
                    nc.sync.dma_start(o_d[ts(t, P), :], o_sb)


def _split_multi_waits(bir: dict) -> dict:
    """The walrus build in this container supports at most ONE sync-wait per
    instruction.  Split extras onto standalone same-engine EventSemaphore
    instructions inserted immediately before — identical semantics."""
    ctr = 0
    for func in bir["functions"]:
        for bb in func["blocks"]:
            new_insts = []
            for inst in bb["instructions"]:
                si = inst.get("sync_info")
                waits = si.get("on_wait") if si else None
                if waits and len(waits) > 1:
                    for w in waits[:-1]:
                        ctr += 1
                        new_insts.append(
                            {
                                "debug": inst.get("debug", 0),
                                "engine": inst["engine"],
                                "ins": [],
                                "outs": [],
                                "name": f"{inst['name']}-wsplit{ctr}",
                                "opcode": "EventSemaphore",
                                "sync_info": {"on_update": [], "on_wait": [w]},
                            }
                        )
                    si["on_wait"] = [waits[-1]]
                new_insts.append(inst)
            bb["instructions"] = new_insts
    return bir


_ORIG_TO_JSON_BYTES = bass.Bass.to_json_bytes


def _patched_to_json_bytes(self) -> bytes:
    bir = json.loads(_ORIG_TO_JSON_BYTES(self))
    _split_multi_waits(bir)
    return json.dumps(bir).encode()


_NC_CACHE = {}


def _build(reps: int = 1) -> bass.Bass:
    key = (reps, tuple(sorted(ABLATE)))
    if key in _NC_CACHE:
        return _NC_CACHE[key]
    nc = bass.Bass(
        "TRN2",
        target_bir_lowering=False,
        debug=False,
        enable_asserts=False,
        num_devices=N_CORES,
    )
    xt_d = nc.dram_tensor(
        "xt_in", [N_TILES, P, KS, P], BF16, kind="ExternalInput").ap()
    wtT_d = nc.dram_tensor(
        "wtT_in", [N_CTRL, B_SHARD], BF16, kind="ExternalInput").ap()
    w_d = nc.dram_tensor(
        "w_in", [N_CTRL, D_IN, D_OUT], BF16, kind="ExternalInput").ap()
    b_d = nc.dram_tensor(
        "b_in", [N_CTRL, D_OUT], BF16, kind="ExternalInput").ap()
    o_d = nc.dram_tensor(
        "out", [B_SHARD, D_OUT], BF16, kind="ExternalOutput").ap()
    import contextlib
    with tile.TileContext(nc) as tc:
        with contextlib.ExitStack() as st:
            gconst = st.enter_context(tc.tile_pool(name="global_const", bufs=1))
            e_pad = _consts(nc, gconst)
            psets = []
            for par in range(min(2, reps)):
                psets.append(tuple(
                    st.enter_context(tc.tile_pool(name=f"{nm}{par}", bufs=bf,
                                                  space=sp))
                    for nm, bf, sp in [
                        ("const", 1, "SBUF"), ("xtpool", 2, "SBUF"),
                        ("xspool", 3, "SBUF"), ("opool", 3, "SBUF"),
                        ("mm_ps", 2, "PSUM"), ("bc_ps", 1, "PSUM"),
                    ]))
            for r in range(reps):
                _body(nc, tc, xt_d, wtT_d, w_d, b_d, o_d, e_pad,
                      psets[r % len(psets)])
    nc.to_json_bytes = types.MethodType(_patched_to_json_bytes, nc)
    _NC_CACHE[key] = nc
    return nc


def _prep_in_maps(inputs, weights, w, b):
    import ml_dtypes

    BF = ml_dtypes.bfloat16
    inputs = np.ascontiguousarray(inputs, dtype=np.float32)
    weights = np.ascontiguousarray(weights, dtype=np.float32)
    w_bf = np.ascontiguousarray(w, dtype=np.float32).astype(BF)
    b_bf = np.ascontiguousarray(b, dtype=np.float32).astype(BF)

    in_maps = []
    for i in range(N_CORES):
        sl = slice(i * B_SHARD, (i + 1) * B_SHARD)
        x_sh = inputs[sl].astype(BF)               # [1024, 512]
        # xt_in[t, p, k, b] = x_sh[t*128 + b, k*128 + p]
        xt = np.ascontiguousarray(
            x_sh.T.reshape(KS, P, N_TILES, P).transpose(2, 1, 0, 3)
        )
        wtT = np.ascontiguousarray(weights[sl].T.astype(BF))   # [8, 1024]
        in_maps.append(
            {"xt_in": xt, "wtT_in": wtT, "w_in": w_bf, "b_in": b_bf}
        )
    return in_maps


def kernel(inputs, weights, w, b, _trace=False):
    nc = _build()
    in_maps = _prep_in_maps(inputs, weights, w, b)
    res = run_bass_kernel_spmd(
        nc, in_maps, core_ids=list(range(N_CORES)), trace=_trace
    )
    out = np.concatenate(
        [r["out"].astype(np.float32) for r in res.results], axis=0
    )
    if _trace:
        return out, res
    return out


# revision 16
# speedup vs baseline: 1.4977x; 1.1786x over previous
"""Mode-adaptive linear (MoE soft routing) Trainium2 kernel, v4.

out[b, o] = sum_c weights[b, c] * (inputs[b, :] @ w[c])[o] + (weights @ bias)[b, o]

The kernel is bound by per-core SBUF bandwidth (~1.2TB/s aggregate across
PE operand streaming, DVE/scalar ops, and DMA writes), so the design
minimizes SBUF bytes touched:

- All inputs staged host-side in bf16: w 8->4MB, x pre-transposed + tiled
  0.5MB, output written bf16 and upcast on host (~5.5MB DMA/core).
- NO scaled-x intermediate and NO routing-weight broadcast: each expert's
  [128x512] PSUM result folds into a per-tile SBUF accumulator on DVE via
  scalar_tensor_tensor (acc = psum * wt[:, t, c] + prev) with the
  natural-layout routing column as the per-partition scalar — the scaling
  reads PSUM, which costs no SBUF bandwidth.
- Blended bias (weights @ b, one K=8 matmul per tile off host-transposed
  wtT) seeds each tile's evacuation chain as the in1 of expert 0's op.
- Expert-major loop: w streams one expert at a time (16 x 256KB DMAs, the
  measured multi-engine sweet spot); per-rep pools are double-buffered so
  back-to-back executions pipeline.
"""

import json
import types

import numpy as np

import concourse.bass as bass
import concourse.mybir as mybir
import concourse.tile as tile
from concourse.bass import ts
from concourse.bass_utils import run_bass_kernel_spmd

N_CORES = 8
ABLATE = set()
B, D_IN, D_OUT, N_CTRL = 8192, 512, 512, 8
B_SHARD = B // N_CORES          # 1024 rows per core
P = 128
N_TILES = B_SHARD // P          # 8 batch tiles per core
KS = D_IN // P                  # 4 K-chunks of 128
F32 = mybir.dt.float32
BF16 = mybir.dt.bfloat16


def _consts(nc: bass.Bass, const_pool):
    """e_pad[p, c, m] = 1 iff p == c: matmul(lhsT=e_pad[0:8, c, :], rhs)
    broadcasts rhs row c to all 128 output partitions (K=8)."""
    import ml_dtypes

    e_np = np.zeros((P, N_CTRL, P), dtype=ml_dtypes.bfloat16)
    for c in range(N_CTRL):
        e_np[c, c, :] = 1.0
    e_d = nc.inline_tensor(e_np, name="e_pad_const")
    e_pad = const_pool.tile([P, N_CTRL, P], BF16)
    nc.sync.dma_start(e_pad, e_d.ap())
    return e_pad


def _body(nc: bass.Bass, tc: tile.TileContext, xt_d, wtT_d, wtn_d, w_d, b_d, o_d,
          e_pad, pools):
    if True:
        (const_pool, xtpool, xspool, opool, mm_psum, bc_psum) = pools
        bc_psum = mm_psum
        # --- Phase 1: small loads; first xt tiles; warmup.  DMAs are
        # deliberately COARSE: per-DMA fixed cost (~1.5us descriptor +
        # semaphore) dominates over bytes on this queue. ---
        xts = []

        def load_xt_half(h):
            blk = xtpool.tile([P, 4, KS, P], BF16, tag="xt")
            nc.sync.dma_start(
                blk, xt_d[4 * h:4 * h + 4].rearrange("t p k b -> p t k b")
            )
            for i in range(4):
                xts.append(blk[:, i])

        load_xt_half(0)

        wtT_sb = const_pool.tile([N_CTRL, B_SHARD], BF16)
        nc.sync.dma_start(wtT_sb, wtT_d)
        wtn_sb = const_pool.tile([P, N_TILES, N_CTRL], F32)
        nc.sync.dma_start(wtn_sb, wtn_d)
        b_sb = const_pool.tile([N_CTRL, D_OUT], BF16)
        nc.sync.dma_start(b_sb, b_d)

        # HAM warmup: dummy bf16 matmuls keep PE executing during the
        # DMA-paced startup so the clock-gate releases early.
        for _ in range(12):
            warm_ps = bc_psum.tile([P, P], F32, tag="bc", bufs=2, name="warm_ps")
            nc.tensor.matmul(
                warm_ps, lhsT=e_pad[:, 0, :], rhs=e_pad[:, 0, :],
                start=True, stop=True,
            )


        # --- Phase 2: expert weights (bf16 in DRAM) straight to SBUF;
        # remaining xt tiles interleaved. ---
        w_sb = const_pool.tile([P, N_CTRL, KS, D_OUT], BF16)

        def load_w(c):
            wr = w_d[c].rearrange("(k p) o -> p k o", p=P)
            nc.sync.dma_start(w_sb[:, c, 0:2], wr[:, 0:2])
            nc.sync.dma_start(w_sb[:, c, 2:4], wr[:, 2:4])

        load_w(0)
        load_w(1)
        load_xt_half(1)
        for c in range(2, N_CTRL):
            load_w(c)

        # --- Phase 3: expert-major evacuation scaling.  Each expert's PSUM
        # result folds into the per-tile SBUF accumulator on DVE:
        # acc = psum * wt[:, t, c] + prev (per-partition scalar, natural
        # layout).  No xs intermediate, no wb broadcast — the scaling reads
        # PSUM, saving ~30MB/rep of SBUF traffic.  Blended bias (K=8 matmul)
        # seeds the chain via in1 of the first expert's evacuation. ---
        MULT = mybir.AluOpType.mult
        ADD = mybir.AluOpType.add
        wtbs = {}
        for t in range(N_TILES):
            wtb_ps = bc_psum.tile([P, D_OUT], F32, tag="bc", bufs=2)
            nc.tensor.matmul(
                wtb_ps, lhsT=wtT_sb[:, ts(t, P)], rhs=b_sb,
                start=True, stop=True,
            )
            wtb = opool.tile([P, D_OUT], F32, tag="wtb", bufs=3)
            nc.scalar.copy(wtb, wtb_ps)
            wtbs[t] = wtb

        accs = [None] * N_TILES
        for c in range(N_CTRL):
            for t in range(N_TILES):
                out_ps = mm_psum.tile([P, D_OUT], F32, tag="acc")
                for k in range(KS):
                    nc.tensor.matmul(
                        out_ps,
                        lhsT=xts[t][:, k, :],
                        rhs=w_sb[:, c, k, :],
                        start=(k == 0),
                        stop=(k == KS - 1),
                    )
                scal = wtn_sb[:, t, c:c + 1]
                if c == 0:
                    acc = opool.tile([P, D_OUT], F32, tag="acc_sb", bufs=N_TILES)
                    nc.vector.scalar_tensor_tensor(
                        acc, out_ps, scal, wtbs.pop(t), op0=MULT, op1=ADD)
                    accs[t] = acc
                elif c < N_CTRL - 1:
                    nc.vector.scalar_tensor_tensor(
                        accs[t], out_ps, scal, accs[t], op0=MULT, op1=ADD)
                else:
                    o_sb = opool.tile([P, D_OUT], BF16, tag="o_sb")
                    nc.vector.scalar_tensor_tensor(
                        o_sb, out_ps, scal, accs[t], op0=MULT, op1=ADD)
                    nc.sync.dma_start(o_d[ts(t, P), :], o_sb)


def _split_multi_waits(bir: dict) -> dict:
    """The walrus build in this container supports at most ONE sync-wait per
    instruction.  Split extras onto standalone same-engine EventSemaphore
    instructions inserted immediately before — identical semantics."""
    ctr = 0
    for func in bir["functions"]:
        for bb in func["blocks"]:
            new_insts = []
            for inst in bb["instructions"]:
                si = inst.get("sync_info")
                waits = si.get("on_wait") if si else None
                if waits and len(waits) > 1:
                    for w in waits[:-1]:
                        ctr += 1
                        new_insts.append(
                            {
                                "debug": inst.get("debug", 0),
                                "engine": inst["engine"],
                                "ins": [],
                                "outs": [],
                                "name": f"{inst['name']}-wsplit{ctr}",
                                "opcode": "EventSemaphore",
                                "sync_info": {"on_update": [], "on_wait": [w]},
                            }
                        )
                    si["on_wait"] = [waits[-1]]
                new_insts.append(inst)
            bb["instructions"] = new_insts
    return bir


_ORIG_TO_JSON_BYTES = bass.Bass.to_json_bytes


def _patched_to_json_bytes(self) -> bytes:
    bir = json.loads(_ORIG_TO_JSON_BYTES(self))
    _split_multi_waits(bir)
    return json.dumps(bir).encode()


_NC_CACHE = {}


def _build(reps: int = 1) -> bass.Bass:
    key = (reps, tuple(sorted(ABLATE)))
    if key in _NC_CACHE:
        return _NC_CACHE[key]
    nc = bass.Bass(
        "TRN2",
        target_bir_lowering=False,
        debug=False,
        enable_asserts=False,
        num_devices=N_CORES,
    )
    xt_d = nc.dram_tensor(
        "xt_in", [N_TILES, P, KS, P], BF16, kind="ExternalInput").ap()
    wtT_d = nc.dram_tensor(
        "wtT_in", [N_CTRL, B_SHARD], BF16, kind="ExternalInput").ap()
    wtn_d = nc.dram_tensor(
        "wtn_in", [P, N_TILES, N_CTRL], F32, kind="ExternalInput").ap()
    w_d = nc.dram_tensor(
        "w_in", [N_CTRL, D_IN, D_OUT], BF16, kind="ExternalInput").ap()
    b_d = nc.dram_tensor(
        "b_in", [N_CTRL, D_OUT], BF16, kind="ExternalInput").ap()
    o_d = nc.dram_tensor(
        "out", [B_SHARD, D_OUT], BF16, kind="ExternalOutput").ap()
    import contextlib
    with tile.TileContext(nc) as tc:
        with contextlib.ExitStack() as st:
            gconst = st.enter_context(tc.tile_pool(name="global_const", bufs=1))
            e_pad = _consts(nc, gconst)
            psets = []
            for par in range(min(2, reps)):
                psets.append(tuple(
                    st.enter_context(tc.tile_pool(name=f"{nm}{par}", bufs=bf,
                                                  space=sp))
                    for nm, bf, sp in [
                        ("const", 1, "SBUF"), ("xtpool", 2, "SBUF"),
                        ("xspool", 3, "SBUF"), ("opool", 3, "SBUF"),
                        ("mm_ps", 2, "PSUM"), ("bc_ps", 1, "PSUM"),
                    ]))
            for r in range(reps):
                _body(nc, tc, xt_d, wtT_d, wtn_d, w_d, b_d, o_d, e_pad,
                      psets[r % len(psets)])
    nc.to_json_bytes = types.MethodType(_patched_to_json_bytes, nc)
    _NC_CACHE[key] = nc
    return nc


def _prep_in_maps(inputs, weights, w, b):
    import ml_dtypes

    BF = ml_dtypes.bfloat16
    inputs = np.ascontiguousarray(inputs, dtype=np.float32)
    weights = np.ascontiguousarray(weights, dtype=np.float32)
    w_bf = np.ascontiguousarray(w, dtype=np.float32).astype(BF)
    b_bf = np.ascontiguousarray(b, dtype=np.float32).astype(BF)

    in_maps = []
    for i in range(N_CORES):
        sl = slice(i * B_SHARD, (i + 1) * B_SHARD)
        x_sh = inputs[sl].astype(BF)               # [1024, 512]
        # xt_in[t, p, k, b] = x_sh[t*128 + b, k*128 + p]
        xt = np.ascontiguousarray(
            x_sh.T.reshape(KS, P, N_TILES, P).transpose(2, 1, 0, 3)
        )
        wtT = np.ascontiguousarray(weights[sl].T.astype(BF))   # [8, 1024]
        # wtn[p, t, c] = weights[sl][t*128 + p, c]
        wtn = np.ascontiguousarray(
            weights[sl].reshape(N_TILES, P, N_CTRL).transpose(1, 0, 2)
        )
        in_maps.append(
            {"xt_in": xt, "wtT_in": wtT, "wtn_in": wtn, "w_in": w_bf,
             "b_in": b_bf}
        )
    return in_maps


def kernel(inputs, weights, w, b, _trace=False):
    nc = _build()
    in_maps = _prep_in_maps(inputs, weights, w, b)
    res = run_bass_kernel_spmd(
        nc, in_maps, core_ids=list(range(N_CORES)), trace=_trace
    )
    out = np.concatenate(
        [r["out"].astype(np.float32) for r in res.results], axis=0
    )
    if _trace:
        return out, res
    return out
